# revision 1
# baseline (speedup 1.0000x reference)
"""Trainium2 Bass kernel for nn_Block_CD (dual-stream patch-embed + attention).

Math per stream (x / y), tokens = (sample, l), l = 25 positions:
  xp  = conv3x3(img) + pos + conv_b           (im2col-ext matmul, K=52)
  xln = (xp - mu) * rstd                      (LN; g/b folded into qkv weights)
  qkv = (16 * qkv_w * g).T @ xln              (feature-major [768, tok])
  scores^T[(h,m),l] = Kbd_s^T @ q_s           (block-diag 4-head groups)
  E = exp(SCALE * scores); den = O^T @ E      (replicated over (h,d))
  av = Vbd_s^T @ E; avn = av / den
  out = proj^T @ avn + xp + (bp - pos - conv_b)
Output device layout: [512, B_loc*25] fp32, rearranged on host.
Sharding: pure data parallel, B=8192 over 8 cores.
"""
import sys
sys.path.insert(0, "/opt/trn_rl_repo")
import numpy as np
import ml_dtypes

import concourse.bass as bass
import concourse.mybir as mybir
import concourse.tile as tile
from concourse import bacc, bass_utils

bf16 = mybir.dt.bfloat16
f32 = mybir.dt.float32
AF = mybir.ActivationFunctionType
ALU = mybir.AluOpType

DIM = 256
HEADS = 8
HD = 32
L = 25
SCALE = HD ** -0.5
LN_EPS = 1e-5
NCORES = 8
B = 8192
B_LOC = B // NCORES

S_O = 64          # samples per outer chunk
S_I = 16          # samples per inner psum slice
N_I = S_I * L     # 400
TOK_O = S_O * L   # 1600

_CACHE = {}


def _to_bf16(a):
    return np.asarray(a, np.float32).astype(ml_dtypes.bfloat16)


def _host_prep(inputs):
    pos = np.asarray(inputs["pos_embed"], np.float32).reshape(L, DIM)
    ln_g = np.asarray(inputs["ln_g"], np.float32)
    ln_b = np.asarray(inputs["ln_b"], np.float32)

    def im2col_ext(img):
        p = np.pad(np.asarray(img, np.float32), ((0, 0), (0, 0), (1, 1), (1, 1)))
        Bn = img.shape[0]
        cols = np.empty((Bn, L, 52), np.float32)
        idx = 0
        for c in range(3):
            for di in range(3):
                for dj in range(3):
                    cols[:, :, idx] = p[:, c, di:di + 5, dj:dj + 5].reshape(Bn, L)
                    idx += 1
        cols[:, :, 27:] = np.eye(L, dtype=np.float32)[None]
        return cols  # [B, 25, 52]

    prep = {}
    for nm, ik, cw, cb, qw, pw, pb in (
        ("x", "x", "conv1_w", "conv1_b", "qkv_x_w", "proj_x_w", "proj_x_b"),
        ("y", "y", "conv2_w", "conv2_b", "qkv_y_w", "proj_y_w", "proj_y_b"),
    ):
        conv_w = np.asarray(inputs[cw], np.float32)
        conv_b = np.asarray(inputs[cb], np.float32)
        qkv_w = np.asarray(inputs[qw], np.float32)
        proj_w = np.asarray(inputs[pw], np.float32)
        proj_b = np.asarray(inputs[pb], np.float32)

        w_emb = np.empty((52, DIM), np.float32)
        w_emb[:27] = conv_w.reshape(DIM, 27).T
        w_emb[27:] = pos + conv_b[None, :]
        prep[f"wemb_{nm}"] = w_emb

        wq = (16.0 * qkv_w * ln_g[None, :]).T          # [256, 768]
        prep[f"wqkv_{nm}"] = np.concatenate([wq[0:128], wq[128:256]], axis=1)  # [128,1536]
        c = qkv_w @ ln_b
        assert np.abs(c).max() < 1e-6, "nonzero ln_b fold not supported"

        wp = proj_w.T                                   # [256, 256] lhsT
        prep[f"wproj_{nm}"] = np.concatenate([wp[0:128], wp[128:256]], axis=1)  # [128,512]

        fb = proj_b[:, None] - pos.T - conv_b[:, None]  # [256, 25]
        fbt = np.tile(fb, (1, S_I))                     # [256, 400]
        prep[f"fb_{nm}"] = np.concatenate([fbt[0:128], fbt[128:256]], axis=1)  # [128,800]

        prep[f"ic_{nm}"] = im2col_ext(inputs[ik])

    prep["ones_ln"] = np.full((128, 128), 1.0 / DIM, np.float32)
    O = np.zeros((128, 128), np.float32)
    for h in range(4):
        O[h * HD:h * HD + L, h * HD:(h + 1) * HD] = 1.0
    prep["ones_den"] = O
    return prep


def _build_kernel(nc, tc, b_loc):
    import contextlib
    ctx = contextlib.ExitStack()
    n_chunk = b_loc // S_O
    n_sl = S_O // S_I

    dram = {}
    for nm in ("x", "y"):
        dram[f"ic_{nm}"] = nc.dram_tensor(f"ic_{nm}", [52, b_loc * L], bf16, kind="ExternalInput").ap()
        dram[f"wemb_{nm}"] = nc.dram_tensor(f"wemb_{nm}", [52, DIM], bf16, kind="ExternalInput").ap()
        dram[f"wqkv_{nm}"] = nc.dram_tensor(f"wqkv_{nm}", [128, 1536], bf16, kind="ExternalInput").ap()
        dram[f"wproj_{nm}"] = nc.dram_tensor(f"wproj_{nm}", [128, 512], bf16, kind="ExternalInput").ap()
        dram[f"fb_{nm}"] = nc.dram_tensor(f"fb_{nm}", [128, 2 * N_I], f32, kind="ExternalInput").ap()
    dram["ones_ln"] = nc.dram_tensor("ones_ln", [128, 128], bf16, kind="ExternalInput").ap()
    dram["ones_den"] = nc.dram_tensor("ones_den", [128, 128], bf16, kind="ExternalInput").ap()
    out_d = nc.dram_tensor("out", [2 * DIM, b_loc * L], f32, kind="ExternalOutput").ap()

    const = ctx.enter_context(tc.tile_pool(name="const", bufs=1))
    sb = ctx.enter_context(tc.tile_pool(name="sb", bufs=1))
    ps = ctx.enter_context(tc.tile_pool(name="ps", bufs=2, space="PSUM"))

    W = {}
    for nm in ("x", "y"):
        for key, shp, dt in (("emb", [52, DIM], bf16), ("qkv", [128, 1536], bf16),
                             ("proj", [128, 512], bf16), ("fb", [128, 2 * N_I], f32)):
            W[f"{key}_{nm}"] = const.tile(shp, dt, tag=f"{key}{nm}", name=f"{key}{nm}")
            nc.sync.dma_start(W[f"{key}_{nm}"][:, :], dram[f"w{key}_{nm}" if key != "fb" else f"fb_{nm}"])
    W["ones_ln"] = const.tile([128, 128], bf16, tag="ones_ln", name="ones_ln")
    nc.sync.dma_start(W["ones_ln"][:, :], dram["ones_ln"])
    W["ones_den"] = const.tile([128, 128], bf16, tag="ones_den", name="ones_den")
    nc.sync.dma_start(W["ones_den"][:, :], dram["ones_den"])
    eps256 = const.tile([128, 1], f32, tag="eps256", name="eps256")
    nc.vector.memset(eps256[:, :], 256.0 * LN_EPS)

    kbd, vbd = {}, {}
    for g in range(2):
        kbd[g] = const.tile([128, 128 * S_O], bf16, tag=f"kbd{g}", name=f"kbd{g}")
        nc.vector.memset(kbd[g][:, :], 0.0)
        vbd[g] = const.tile([128, 128 * S_O], bf16, tag=f"vbd{g}", name=f"vbd{g}")
        nc.vector.memset(vbd[g][:, :], 0.0)

    for ci in range(n_chunk):
        for nm in ("x", "y"):
            tok0 = ci * TOK_O
            # ---- embed ----
            ic = sb.tile([52, TOK_O], bf16, tag="ic", bufs=2)
            nc.sync.dma_start(ic[:, :], dram[f"ic_{nm}"][:, tok0:tok0 + TOK_O])
            xp = [sb.tile([128, TOK_O], bf16, tag=f"xp{t}", name=f"xp{t}") for t in range(2)]
            xpf = [sb.tile([128, TOK_O], f32, tag=f"xpf{t}", name=f"xpf{t}") for t in range(2)]
            sq = [sb.tile([128, TOK_O], bf16, tag=f"sq{t}", name=f"sq{t}") for t in range(2)]
            for t in range(2):
                for s in range(n_sl):
                    pt = ps.tile([128, N_I], f32, tag="mm", bufs=3)
                    nc.tensor.matmul(pt[:, :], W[f"emb_{nm}"][:, 128 * t:128 * (t + 1)],
                                     ic[:, s * N_I:(s + 1) * N_I], start=True, stop=True)
                    sl = slice(s * N_I, (s + 1) * N_I)
                    nc.scalar.activation(xpf[t][:, sl], pt[:, :], AF.Copy)
                    nc.vector.tensor_copy(xp[t][:, sl], pt[:, :])
                    nc.scalar.activation(sq[t][:, sl], pt[:, :], AF.Square)
            # ---- LN stats ----
            mu = sb.tile([128, TOK_O], f32, tag="mu")
            rs = sb.tile([128, TOK_O], f32, tag="rs")
            for s in range(n_sl):
                sl = slice(s * N_I, (s + 1) * N_I)
                pm = ps.tile([128, N_I], f32, tag="mm", bufs=3)
                nc.tensor.matmul(pm[:, :], W["ones_ln"][:, :], xp[0][:, sl], start=True, stop=False)
                nc.tensor.matmul(pm[:, :], W["ones_ln"][:, :], xp[1][:, sl], start=False, stop=True)
                pv = ps.tile([128, N_I], f32, tag="mm", bufs=3)
                nc.tensor.matmul(pv[:, :], W["ones_ln"][:, :], sq[0][:, sl], start=True, stop=False)
                nc.tensor.matmul(pv[:, :], W["ones_ln"][:, :], sq[1][:, sl], start=False, stop=True)
                nc.scalar.activation(mu[:, sl], pm[:, :], AF.Copy)
                t1 = sb.tile([128, N_I], f32, tag="t1")
                nc.scalar.activation(t1[:, :], pm[:, :], AF.Square)
                nc.vector.tensor_sub(t1[:, :], pv[:, :], t1[:, :])
                t2 = sb.tile([128, N_I], f32, tag="t2")
                nc.scalar.activation(t2[:, :], t1[:, :], AF.Sqrt, bias=eps256[:, 0:1], scale=256.0)
                nc.vector.reciprocal_approx_fast(rs[:, sl], t2[:, :])
            # ---- LN apply ----
            xln = [sb.tile([128, TOK_O], bf16, tag=f"xln{t}", name=f"xln{t}") for t in range(2)]
            for t in range(2):
                d = sb.tile([128, TOK_O], bf16, tag="lnd")
                nc.vector.tensor_sub(d[:, :], xp[t][:, :], mu[:, :])
                nc.vector.tensor_mul(xln[t][:, :], d[:, :], rs[:, :])
            # ---- qkv ----
            qkv = [sb.tile([128, TOK_O], bf16, tag=f"qkv{m}", name=f"qkv{m}") for m in range(4)]
            qkv += [sb.tile([128, 32 * S_O], bf16, tag=f"qkv{m}", name=f"qkv{m}") for m in (4, 5)]
            for m in (4, 5):
                nc.gpsimd.memset(
                    qkv[m][:, :].rearrange("p (s l) -> p s l", l=32)[:, :, L:32], 0.0)
            for m in range(6):
                for s in range(n_sl):
                    sl = slice(s * N_I, (s + 1) * N_I)
                    pq = ps.tile([128, N_I], f32, tag="mm", bufs=3)
                    nc.tensor.matmul(pq[:, :], W[f"qkv_{nm}"][:, 128 * m:128 * (m + 1)],
                                     xln[0][:, sl], start=True, stop=False)
                    nc.tensor.matmul(pq[:, :], W[f"qkv_{nm}"][:, 768 + 128 * m:768 + 128 * (m + 1)],
                                     xln[1][:, sl], start=False, stop=True)
                    if m < 4:
                        nc.vector.tensor_copy(qkv[m][:, sl], pq[:, :])
                    else:
                        nc.vector.tensor_copy(
                            qkv[m][:, :].rearrange("p (s l) -> p s l", l=32)[:, s * S_I:(s + 1) * S_I, 0:L],
                            pq[:, :].rearrange("p (s l) -> p s l", l=L))
            # ---- attention ----
            vt = [sb.tile([128, 32 * S_O], bf16, tag=f"vt{g}", name=f"vt{g}") for g in range(2)]
            for g in range(2):
                nc.vector.transpose(vt[g][:, :], qkv[4 + g][:, :])
            for g in range(2):
                for h in range(4):
                    nc.scalar.activation(
                        kbd[g][32 * h:32 * h + 32, :]
                        .rearrange("p (s m) -> p s m", m=128)[:, :, 32 * h:32 * h + 25],
                        qkv[2 + g][32 * h:32 * h + 32, :]
                        .rearrange("p (s m) -> p s m", m=L), AF.Copy)
                    nc.vector.tensor_copy(
                        vbd[g][32 * h:32 * h + 25, :]
                        .rearrange("p (s d) -> p s d", d=128)[:, :, 32 * h:32 * h + 32],
                        vt[g][32 * h:32 * h + 25, :]
                        .rearrange("p (s d) -> p s d", d=32))
            ebuf = [sb.tile([128, L * S_O], bf16, tag=f"e{g}", name=f"e{g}") for g in range(2)]
            avn = [sb.tile([128, L * S_O], bf16, tag=f"avn{g}", name=f"avn{g}") for g in range(2)]
            for g in range(2):
                for s in range(n_sl):
                    sl = slice(s * N_I, (s + 1) * N_I)
                    sc = ps.tile([128, N_I], f32, tag="sc", bufs=3)
                    for j in range(S_I):
                        si = s * S_I + j
                        nc.tensor.matmul(
                            sc[0:128, j * L:(j + 1) * L],
                            kbd[g][:, 128 * si:128 * (si + 1)],
                            qkv[g][:, L * si:L * (si + 1)],
                            start=True, stop=True)
                    nc.scalar.activation(ebuf[g][:, sl], sc[:, :], AF.Exp, scale=SCALE)
                    dn = ps.tile([128, N_I], f32, tag="mm", bufs=3)
                    nc.tensor.matmul(dn[:, :], W["ones_den"][:, :], ebuf[g][:, sl],
                                     start=True, stop=True)
                    rden = sb.tile([128, N_I], f32, tag="rden", bufs=2)
                    nc.vector.reciprocal_approx_fast(rden[:, :], dn[:, :])
                    av = ps.tile([128, N_I], f32, tag="av")
                    for j in range(S_I):
                        si = s * S_I + j
                        nc.tensor.matmul(
                            av[:, j * L:(j + 1) * L],
                            vbd[g][:, 128 * si:128 * (si + 1)],
                            ebuf[g][:, L * si:L * (si + 1)],
                            start=True, stop=True)
                    nc.vector.tensor_mul(avn[g][:, sl], av[:, :], rden[:, :])
            # ---- proj + residual + out ----
            ob = 0 if nm == "x" else DIM
            for t in range(2):
                for s in range(n_sl):
                    sl = slice(s * N_I, (s + 1) * N_I)
                    pp = ps.tile([128, N_I], f32, tag="mm", bufs=3)
                    nc.tensor.matmul(pp[:, :], W[f"proj_{nm}"][:, 128 * t:128 * (t + 1)],
                                     avn[0][:, sl], start=True, stop=False)
                    nc.tensor.matmul(pp[:, :], W[f"proj_{nm}"][:, 256 + 128 * t:256 + 128 * (t + 1)],
                                     avn[1][:, sl], start=False, stop=True)
                    o1 = sb.tile([128, N_I], f32, tag="o1")
                    nc.vector.tensor_add(o1[:, :], pp[:, :], xpf[t][:, sl])
                    o2 = sb.tile([128, N_I], f32, tag="o2", bufs=2)
                    nc.gpsimd.tensor_add(o2[:, :], o1[:, :],
                                         W[f"fb_{nm}"][:, N_I * t:N_I * (t + 1)])
                    nc.sync.dma_start(
                        out_d[ob + 128 * t: ob + 128 * (t + 1),
                              tok0 + s * N_I: tok0 + (s + 1) * N_I],
                        o2[:, :])
    ctx.close()


def _get_nc(b_loc):
    if b_loc in _CACHE:
        return _CACHE[b_loc]
    nc = bacc.Bacc("TRN2", target_bir_lowering=False, debug=False,
                   enable_asserts=False, num_devices=NCORES)
    with tile.TileContext(nc, trace_sim=False) as tc:
        _build_kernel(nc, tc, b_loc)
    nc.compile()
    bass.Bass.finalize(nc)
    _CACHE[b_loc] = nc
    return nc


def _in_maps(prep, b_loc, ncores):
    maps = []
    for c in range(ncores):
        s0 = c * b_loc
        m = {}
        for nm in ("x", "y"):
            ic = prep[f"ic_{nm}"][s0:s0 + b_loc].reshape(b_loc * L, 52).T
            m[f"ic_{nm}"] = _to_bf16(np.ascontiguousarray(ic))
            m[f"wemb_{nm}"] = _to_bf16(prep[f"wemb_{nm}"])
            m[f"wqkv_{nm}"] = _to_bf16(prep[f"wqkv_{nm}"])
            m[f"wproj_{nm}"] = _to_bf16(prep[f"wproj_{nm}"])
            m[f"fb_{nm}"] = prep[f"fb_{nm}"].astype(np.float32)
        m["ones_ln"] = _to_bf16(prep["ones_ln"])
        m["ones_den"] = _to_bf16(prep["ones_den"])
        maps.append(m)
    return maps


def kernel(**inputs):
    prep = _host_prep(inputs)
    nc = _get_nc(B_LOC)
    res = bass_utils.run_bass_kernel_spmd(nc, _in_maps(prep, B_LOC, NCORES),
                                          core_ids=list(range(NCORES)))
    outs = [res.results[c]["out"] for c in range(NCORES)]
    full = np.concatenate(
        [np.asarray(o, np.float32).reshape(2 * DIM, B_LOC, L).transpose(1, 0, 2)
         for o in outs], axis=0)
    return np.ascontiguousarray(full.reshape(B, 2 * DIM, 5, 5))



# revision 6
# speedup vs baseline: 1094.1991x; 1094.1991x over previous
"""Trainium2 Bass kernel for nn_Block_CD (dual-stream patch-embed + attention).

Math per stream (x / y), tokens = (sample, l), l = 25 positions:
  xp  = conv3x3(img) + pos + conv_b           (im2col-ext matmul, K=52)
  xln = (xp - mu) * rstd                      (LN; g/b folded into qkv weights)
  qkv = (16 * qkv_w * g).T @ xln              (feature-major [768, tok])
  scores^T[(h,m),l] = Kbd_s^T @ q_s           (block-diag 4-head groups)
  E = exp(SCALE * scores); den = O^T @ E      (replicated over (h,d))
  av = Vbd_s^T @ E; avn = av / den
  out = proj^T @ avn + xp + (bp - pos - conv_b)
Output device layout: [512, B_loc*25] bf16, rearranged on host.
Sharding: pure data parallel, B=8192 over 8 cores.

v2: hardware For_i loop over token chunks (2 slices of 16 samples per
iteration) instead of a fully unrolled python loop — cuts the BIR from
~33k instructions to ~700, which removes the dominant per-call host
lowering cost and per-instruction dispatch overhead.
"""
import sys
sys.path.insert(0, "/opt/trn_rl_repo")
import numpy as np
import ml_dtypes

import concourse.bass as bass
import concourse.mybir as mybir
import concourse.tile as tile
from concourse import bacc, bass_utils
from concourse.bass import ds

bf16 = mybir.dt.bfloat16
f32 = mybir.dt.float32
AF = mybir.ActivationFunctionType
ALU = mybir.AluOpType

DIM = 256
HEADS = 8
HD = 32
L = 25
SCALE = HD ** -0.5
LN_EPS = 1e-5
NCORES = 8
B = 8192
B_LOC = B // NCORES

S_I = 16          # samples per inner psum slice
N_I = S_I * L     # 400
U = 2             # slices unrolled per hw-loop iteration
TOK_B = U * N_I   # 800 tokens per iteration

_CACHE = {}


def _to_bf16(a):
    return np.asarray(a, np.float32).astype(ml_dtypes.bfloat16)


def _host_prep(inputs):
    pos = np.asarray(inputs["pos_embed"], np.float32).reshape(L, DIM)
    ln_g = np.asarray(inputs["ln_g"], np.float32)
    ln_b = np.asarray(inputs["ln_b"], np.float32)

    def im2col_ext(img):
        p = np.pad(np.asarray(img, np.float32), ((0, 0), (0, 0), (1, 1), (1, 1)))
        Bn = img.shape[0]
        cols = np.empty((Bn, L, 52), np.float32)
        idx = 0
        for c in range(3):
            for di in range(3):
                for dj in range(3):
                    cols[:, :, idx] = p[:, c, di:di + 5, dj:dj + 5].reshape(Bn, L)
                    idx += 1
        cols[:, :, 27:] = np.eye(L, dtype=np.float32)[None]
        return cols  # [B, 25, 52]

    prep = {}
    for nm, ik, cw, cb, qw, pw, pb in (
        ("x", "x", "conv1_w", "conv1_b", "qkv_x_w", "proj_x_w", "proj_x_b"),
        ("y", "y", "conv2_w", "conv2_b", "qkv_y_w", "proj_y_w", "proj_y_b"),
    ):
        conv_w = np.asarray(inputs[cw], np.float32)
        conv_b = np.asarray(inputs[cb], np.float32)
        qkv_w = np.asarray(inputs[qw], np.float32)
        proj_w = np.asarray(inputs[pw], np.float32)
        proj_b = np.asarray(inputs[pb], np.float32)

        w_emb = np.empty((52, DIM), np.float32)
        w_emb[:27] = conv_w.reshape(DIM, 27).T
        w_emb[27:] = pos + conv_b[None, :]
        prep[f"wemb_{nm}"] = w_emb

        wq = (16.0 * qkv_w * ln_g[None, :]).T          # [256, 768]
        prep[f"wqkv_{nm}"] = np.concatenate([wq[0:128], wq[128:256]], axis=1)  # [128,1536]
        c = qkv_w @ ln_b
        assert np.abs(c).max() < 1e-6, "nonzero ln_b fold not supported"

        wp = proj_w.T                                   # [256, 256] lhsT
        prep[f"wproj_{nm}"] = np.concatenate([wp[0:128], wp[128:256]], axis=1)  # [128,512]

        fb = proj_b[:, None] - pos.T - conv_b[:, None]  # [256, 25]
        fbt = np.tile(fb, (1, S_I))                     # [256, 400]
        prep[f"fb_{nm}"] = np.concatenate([fbt[0:128], fbt[128:256]], axis=1)  # [128,800]

        prep[f"ic_{nm}"] = im2col_ext(inputs[ik])

    prep["ones_ln"] = np.full((128, 128), 1.0 / DIM, np.float32)
    O = np.zeros((128, 128), np.float32)
    for h in range(4):
        O[h * HD:h * HD + L, h * HD:(h + 1) * HD] = 1.0
    prep["ones_den"] = O
    return prep


def _slice_body(nc, sb, ps, W, nm, ic, u, kbd, vbd, o2):
    """Process one 16-sample slice; ic cols [u*400,(u+1)*400), write o2[t][:, u*400...]."""
    cl = slice(u * N_I, (u + 1) * N_I)
    # ---- embed ----
    xp = [sb.tile([128, N_I], bf16, tag=f"xp{t}{u}", name=f"xp{t}{u}") for t in range(2)]
    xpf = [sb.tile([128, N_I], f32, tag=f"xpf{t}{u}", name=f"xpf{t}{u}") for t in range(2)]
    sq = [sb.tile([128, N_I], bf16, tag=f"sq{t}{u}", name=f"sq{t}{u}") for t in range(2)]
    for t in range(2):
        pt = ps.tile([128, N_I], f32, tag="mm", bufs=3)
        nc.tensor.matmul(pt[:, :], W[f"emb_{nm}"][:, 128 * t:128 * (t + 1)],
                         ic[:, cl], start=True, stop=True)
        nc.scalar.activation(xpf[t][:, :], pt[:, :], AF.Copy)
        nc.vector.tensor_copy(xp[t][:, :], pt[:, :])
        nc.scalar.activation(sq[t][:, :], pt[:, :], AF.Square)
    # ---- LN stats ----
    rs = sb.tile([128, N_I], f32, tag=f"rs{u}")
    mu = sb.tile([128, N_I], f32, tag=f"mu{u}")
    pm = ps.tile([128, N_I], f32, tag="mm", bufs=3)
    nc.tensor.matmul(pm[:, :], W["ones_ln"][:, :], xp[0][:, :], start=True, stop=False)
    nc.tensor.matmul(pm[:, :], W["ones_ln"][:, :], xp[1][:, :], start=False, stop=True)
    pv = ps.tile([128, N_I], f32, tag="mm", bufs=3)
    nc.tensor.matmul(pv[:, :], W["ones_ln"][:, :], sq[0][:, :], start=True, stop=False)
    nc.tensor.matmul(pv[:, :], W["ones_ln"][:, :], sq[1][:, :], start=False, stop=True)
    nc.scalar.activation(mu[:, :], pm[:, :], AF.Copy)
    t1 = sb.tile([128, N_I], f32, tag=f"t1{u}")
    nc.scalar.activation(t1[:, :], pm[:, :], AF.Square)
    nc.vector.tensor_sub(t1[:, :], pv[:, :], t1[:, :])
    t2 = sb.tile([128, N_I], f32, tag=f"t2{u}")
    nc.scalar.activation(t2[:, :], t1[:, :], AF.Sqrt, bias=W["eps256"][:, 0:1], scale=256.0)
    nc.vector.reciprocal_approx_fast(rs[:, :], t2[:, :])
    # ---- LN apply ----
    xln = [sb.tile([128, N_I], bf16, tag=f"xln{t}{u}", name=f"xln{t}{u}") for t in range(2)]
    for t in range(2):
        d = sb.tile([128, N_I], bf16, tag=f"lnd{u}")
        nc.vector.tensor_sub(d[:, :], xp[t][:, :], mu[:, :])
        nc.vector.tensor_mul(xln[t][:, :], d[:, :], rs[:, :])
    # ---- qkv ----
    qkv = [sb.tile([128, N_I], bf16, tag=f"qkv{m}{u}", name=f"qkv{m}{u}") for m in range(4)]
    qkv += [sb.tile([128, 32 * S_I], bf16, tag=f"qkv{m}{u}", name=f"qkvv{m}{u}") for m in (4, 5)]
    for m in (4, 5):
        nc.gpsimd.memset(
            qkv[m][:, :].rearrange("p (s l) -> p s l", l=32)[:, :, L:32], 0.0)
    for m in range(6):
        pq = ps.tile([128, N_I], f32, tag="mm", bufs=3)
        nc.tensor.matmul(pq[:, :], W[f"qkv_{nm}"][:, 128 * m:128 * (m + 1)],
                         xln[0][:, :], start=True, stop=False)
        nc.tensor.matmul(pq[:, :], W[f"qkv_{nm}"][:, 768 + 128 * m:768 + 128 * (m + 1)],
                         xln[1][:, :], start=False, stop=True)
        if m < 4:
            nc.vector.tensor_copy(qkv[m][:, :], pq[:, :])
        else:
            nc.vector.tensor_copy(
                qkv[m][:, :].rearrange("p (s l) -> p s l", l=32)[:, :, 0:L],
                pq[:, :].rearrange("p (s l) -> p s l", l=L))
    # ---- block-diag K / V builds ----
    vt = [sb.tile([128, 32 * S_I], bf16, tag=f"vt{g}{u}", name=f"vt{g}{u}") for g in range(2)]
    for g in range(2):
        nc.vector.transpose(vt[g][:, :], qkv[4 + g][:, :])
    for g in range(2):
        for h in range(4):
            nc.scalar.activation(
                kbd[g][32 * h:32 * h + 32, :]
                .rearrange("p (s m) -> p s m", m=128)[:, :, 32 * h:32 * h + L],
                qkv[2 + g][32 * h:32 * h + 32, :]
                .rearrange("p (s m) -> p s m", m=L), AF.Copy)
            nc.vector.tensor_copy(
                vbd[g][32 * h:32 * h + L, :]
                .rearrange("p (s d) -> p s d", d=128)[:, :, 32 * h:32 * h + 32],
                vt[g][32 * h:32 * h + L, :]
                .rearrange("p (s d) -> p s d", d=32))
    # ---- attention ----
    avn = [sb.tile([128, N_I], bf16, tag=f"avn{g}{u}", name=f"avn{g}{u}") for g in range(2)]
    for g in range(2):
        sc = ps.tile([128, N_I], f32, tag="sc", bufs=2)
        for j in range(S_I):
            nc.tensor.matmul(
                sc[0:128, j * L:(j + 1) * L],
                kbd[g][:, 128 * j:128 * (j + 1)],
                qkv[g][:, L * j:L * (j + 1)],
                start=True, stop=True)
        ebuf = sb.tile([128, N_I], bf16, tag=f"e{g}{u}")
        nc.scalar.activation(ebuf[:, :], sc[:, :], AF.Exp, scale=SCALE)
        dn = ps.tile([128, N_I], f32, tag="mm", bufs=3)
        nc.tensor.matmul(dn[:, :], W["ones_den"][:, :], ebuf[:, :],
                         start=True, stop=True)
        rden = sb.tile([128, N_I], f32, tag=f"rden{u}", bufs=2)
        nc.vector.reciprocal_approx_fast(rden[:, :], dn[:, :])
        av = ps.tile([128, N_I], f32, tag="sc", bufs=2)
        for j in range(S_I):
            nc.tensor.matmul(
                av[:, j * L:(j + 1) * L],
                vbd[g][:, 128 * j:128 * (j + 1)],
                ebuf[:, L * j:L * (j + 1)],
                start=True, stop=True)
        nc.vector.tensor_mul(avn[g][:, :], av[:, :], rden[:, :])
    # ---- proj + residual ----
    for t in range(2):
        pp = ps.tile([128, N_I], f32, tag="mm", bufs=3)
        nc.tensor.matmul(pp[:, :], W[f"proj_{nm}"][:, 128 * t:128 * (t + 1)],
                         avn[0][:, :], start=True, stop=False)
        nc.tensor.matmul(pp[:, :], W[f"proj_{nm}"][:, 256 + 128 * t:256 + 128 * (t + 1)],
                         avn[1][:, :], start=False, stop=True)
        o1 = sb.tile([128, N_I], f32, tag=f"o1{t}{u}")
        nc.vector.tensor_add(o1[:, :], pp[:, :], xpf[t][:, :])
        nc.gpsimd.tensor_add(o2[t][:, cl], o1[:, :],
                             W[f"fb_{nm}"][:, N_I * t:N_I * (t + 1)])


def _build_kernel(nc, tc, b_loc, loop_tok=None, static_dma=False):
    import contextlib
    ctx = contextlib.ExitStack()
    n_tok = b_loc * L
    if loop_tok is None:
        loop_tok = n_tok

    dram = {}
    for nm in ("x", "y"):
        dram[f"ic_{nm}"] = nc.dram_tensor(f"ic_{nm}", [52, n_tok], bf16, kind="ExternalInput").ap()
        dram[f"wemb_{nm}"] = nc.dram_tensor(f"wemb_{nm}", [52, DIM], bf16, kind="ExternalInput").ap()
        dram[f"wqkv_{nm}"] = nc.dram_tensor(f"wqkv_{nm}", [128, 1536], bf16, kind="ExternalInput").ap()
        dram[f"wproj_{nm}"] = nc.dram_tensor(f"wproj_{nm}", [128, 512], bf16, kind="ExternalInput").ap()
        dram[f"fb_{nm}"] = nc.dram_tensor(f"fb_{nm}", [128, 2 * N_I], f32, kind="ExternalInput").ap()
    dram["ones_ln"] = nc.dram_tensor("ones_ln", [128, 128], bf16, kind="ExternalInput").ap()
    dram["ones_den"] = nc.dram_tensor("ones_den", [128, 128], bf16, kind="ExternalInput").ap()
    out_d = nc.dram_tensor("out", [2 * DIM, n_tok], bf16, kind="ExternalOutput").ap()

    const = ctx.enter_context(tc.tile_pool(name="const", bufs=1))
    sb = ctx.enter_context(tc.tile_pool(name="sb", bufs=2))
    ps = ctx.enter_context(tc.tile_pool(name="ps", bufs=2, space="PSUM"))

    W = {}
    for nm in ("x", "y"):
        for key, shp, dt in (("emb", [52, DIM], bf16), ("qkv", [128, 1536], bf16),
                             ("proj", [128, 512], bf16), ("fb", [128, 2 * N_I], f32)):
            W[f"{key}_{nm}"] = const.tile(shp, dt, tag=f"{key}{nm}", name=f"{key}{nm}")
            nc.sync.dma_start(W[f"{key}_{nm}"][:, :], dram[f"w{key}_{nm}" if key != "fb" else f"fb_{nm}"])
    W["ones_ln"] = const.tile([128, 128], bf16, tag="ones_ln", name="ones_ln")
    nc.sync.dma_start(W["ones_ln"][:, :], dram["ones_ln"])
    W["ones_den"] = const.tile([128, 128], bf16, tag="ones_den", name="ones_den")
    nc.sync.dma_start(W["ones_den"][:, :], dram["ones_den"])
    W["eps256"] = const.tile([128, 1], f32, tag="eps256", name="eps256")
    nc.vector.memset(W["eps256"][:, :], 256.0 * LN_EPS)

    # block-diag staging tiles: preamble-zeroed once; loop bodies overwrite
    # only the in-block 25/32-col regions, padding stays zero.
    kbd, vbd = {}, {}
    for u in range(U):
        for g in range(2):
            kbd[(g, u)] = const.tile([128, 128 * S_I], bf16, tag=f"kbd{g}{u}", name=f"kbd{g}{u}")
            nc.vector.memset(kbd[(g, u)][:, :], 0.0)
            vbd[(g, u)] = const.tile([128, 128 * S_I], bf16, tag=f"vbd{g}{u}", name=f"vbd{g}{u}")
            nc.vector.memset(vbd[(g, u)][:, :], 0.0)

    for nm in ("x", "y"):
        ob = 0 if nm == "x" else DIM
        with tc.For_i(0, loop_tok, TOK_B, name=f"chunks_{nm}") as tok0:
            ic = sb.tile([52, TOK_B], bf16, tag="ic", bufs=2)
            if static_dma:
                nc.sync.dma_start(ic[:, :], dram[f"ic_{nm}"][:, 0:TOK_B])
            else:
                nc.sync.dma_start(ic[:, :], dram[f"ic_{nm}"][:, ds(tok0, TOK_B)])
            o2 = [sb.tile([128, TOK_B], bf16, tag=f"o2{t}", bufs=2, name=f"o2{t}") for t in range(2)]
            for u in range(U):
                _slice_body(nc, sb, ps, W, nm, ic, u,
                            [kbd[(0, u)], kbd[(1, u)]], [vbd[(0, u)], vbd[(1, u)]], o2)
            for t in range(2):
                if static_dma:
                    nc.sync.dma_start(out_d[ob + 128 * t: ob + 128 * (t + 1), 0:TOK_B],
                                      o2[t][:, :])
                else:
                    nc.sync.dma_start(out_d[ob + 128 * t: ob + 128 * (t + 1), ds(tok0, TOK_B)],
                                      o2[t][:, :])
    ctx.close()


def _get_nc(b_loc, loop_tok=None, static_dma=False):
    key = (b_loc, loop_tok, static_dma)
    if key in _CACHE:
        return _CACHE[key]
    nc = bacc.Bacc("TRN2", target_bir_lowering=False, debug=False,
                   enable_asserts=False, num_devices=NCORES)
    with tile.TileContext(nc, trace_sim=False) as tc:
        _build_kernel(nc, tc, b_loc, loop_tok, static_dma)
    nc.compile()
    bass.Bass.finalize(nc)
    _CACHE[key] = nc
    return nc


def _in_maps(prep, b_loc, ncores):
    maps = []
    for c in range(ncores):
        s0 = c * b_loc
        m = {}
        for nm in ("x", "y"):
            ic = prep[f"ic_{nm}"][s0:s0 + b_loc].reshape(b_loc * L, 52).T
            m[f"ic_{nm}"] = _to_bf16(np.ascontiguousarray(ic))
            m[f"wemb_{nm}"] = _to_bf16(prep[f"wemb_{nm}"])
            m[f"wqkv_{nm}"] = _to_bf16(prep[f"wqkv_{nm}"])
            m[f"wproj_{nm}"] = _to_bf16(prep[f"wproj_{nm}"])
            m[f"fb_{nm}"] = prep[f"fb_{nm}"].astype(np.float32)
        m["ones_ln"] = _to_bf16(prep["ones_ln"])
        m["ones_den"] = _to_bf16(prep["ones_den"])
        maps.append(m)
    return maps


def kernel(**inputs):
    prep = _host_prep(inputs)
    nc = _get_nc(B_LOC)
    res = bass_utils.run_bass_kernel_spmd(nc, _in_maps(prep, B_LOC, NCORES),
                                          core_ids=list(range(NCORES)))
    outs = [res.results[c]["out"] for c in range(NCORES)]
    full = np.concatenate(
        [np.asarray(o, np.float32).reshape(2 * DIM, B_LOC, L).transpose(1, 0, 2)
         for o in outs], axis=0)
    return np.ascontiguousarray(full.reshape(B, 2 * DIM, 5, 5))


# revision 18
# speedup vs baseline: 1737.6213x; 1.5880x over previous
"""Trainium2 Bass kernel for nn_Block_CD (dual-stream patch-embed + attention).

Math per stream (x / y), tokens = (sample, l), l = 25 positions:
  xp  = conv3x3(img) + pos + conv_b           (im2col-ext matmul, K=52)
  xln = (xp - mu) * rstd                      (LN; g/b folded into qkv weights)
  qkv = (16 * qkv_w * g).T @ xln              (feature-major [768, tok])
  scores^T[(h,m),l] = Kbd_s^T @ q_s           (block-diag 4-head groups)
  E = exp(SCALE * scores); den = O^T @ E      (replicated over (h,d))
  av = Vbd_s^T @ E; avn = av / den
  out = proj^T @ avn + xp + (bp - pos - conv_b)
Output device layout: [512, B_loc*25] bf16, rearranged on host.
Sharding: pure data parallel, B=8192 over 8 cores.

v2: hardware For_i loop over token chunks (2 slices of 16 samples per
iteration) instead of a fully unrolled python loop — cuts the BIR from
~33k instructions to ~700, which removes the dominant per-call host
lowering cost and per-instruction dispatch overhead.
"""
import sys
sys.path.insert(0, "/opt/trn_rl_repo")
import numpy as np
import ml_dtypes

import concourse.bass as bass
import concourse.mybir as mybir
import concourse.tile as tile
from concourse import bacc, bass_utils
from concourse.bass import ds

bf16 = mybir.dt.bfloat16
f32 = mybir.dt.float32
AF = mybir.ActivationFunctionType
ALU = mybir.AluOpType

DIM = 256
HEADS = 8
HD = 32
L = 25
SCALE = HD ** -0.5
LN_EPS = 1e-5
NCORES = 8
B = 8192
B_LOC = B // NCORES

S_I = 16          # samples per inner psum slice
N_I = S_I * L     # 400
U = 4             # slices unrolled per hw-loop iteration
STAGGER = 1       # phase offset between consecutive slices (sw pipeline)
TOK_B = U * N_I   # 800 tokens per iteration

_CACHE = {}


def _to_bf16(a):
    return np.asarray(a, np.float32).astype(ml_dtypes.bfloat16)


def _host_prep(inputs):
    pos = np.asarray(inputs["pos_embed"], np.float32).reshape(L, DIM)
    ln_g = np.asarray(inputs["ln_g"], np.float32)
    ln_b = np.asarray(inputs["ln_b"], np.float32)

    def im2col_ext(img):
        p = np.pad(np.asarray(img, np.float32), ((0, 0), (0, 0), (1, 1), (1, 1)))
        Bn = img.shape[0]
        cols = np.empty((Bn, L, 52), np.float32)
        idx = 0
        for c in range(3):
            for di in range(3):
                for dj in range(3):
                    cols[:, :, idx] = p[:, c, di:di + 5, dj:dj + 5].reshape(Bn, L)
                    idx += 1
        cols[:, :, 27:] = np.eye(L, dtype=np.float32)[None]
        return cols  # [B, 25, 52]

    prep = {}
    for nm, ik, cw, cb, qw, pw, pb in (
        ("x", "x", "conv1_w", "conv1_b", "qkv_x_w", "proj_x_w", "proj_x_b"),
        ("y", "y", "conv2_w", "conv2_b", "qkv_y_w", "proj_y_w", "proj_y_b"),
    ):
        conv_w = np.asarray(inputs[cw], np.float32)
        conv_b = np.asarray(inputs[cb], np.float32)
        qkv_w = np.asarray(inputs[qw], np.float32)
        proj_w = np.asarray(inputs[pw], np.float32)
        proj_b = np.asarray(inputs[pb], np.float32)

        w_emb = np.empty((52, DIM), np.float32)
        w_emb[:27] = conv_w.reshape(DIM, 27).T
        w_emb[27:] = pos + conv_b[None, :]
        prep[f"wemb_{nm}"] = w_emb

        wq = (16.0 * qkv_w * ln_g[None, :]).T          # [256, 768]
        w_emb_c = w_emb - w_emb.mean(axis=1, keepdims=True)
        prep[f"wqkv_{nm}"] = w_emb_c @ wq              # [52, 768] (LN mean folded)
        c = qkv_w @ ln_b
        assert np.abs(c).max() < 1e-6, "nonzero ln_b fold not supported"

        wp = proj_w.T                                   # [256, 256] lhsT
        prep[f"wproj_{nm}"] = np.concatenate([wp[0:128], wp[128:256]], axis=1)  # [128,512]

        fb = proj_b[:, None] - pos.T - conv_b[:, None]  # [256, 25]
        fbt = np.tile(fb, (1, S_I))                     # [256, 400]
        prep[f"fb_{nm}"] = np.concatenate([fbt[0:128], fbt[128:256]], axis=1)  # [128,800]

        prep[f"ic_{nm}"] = im2col_ext(inputs[ik])

    prep["ones_ln"] = np.full((128, 128), 1.0 / DIM, np.float32)
    O = np.zeros((128, 128), np.float32)
    for h in range(4):
        O[h * HD:h * HD + L, h * HD:(h + 1) * HD] = 1.0
    prep["ones_den"] = O
    return prep


def _slice_phases(nc, sb, ps, W, nm, ic, u, kbd, vbd, o2):
    """Return a list of phase-emitter closures for one 16-sample slice."""
    st = {}

    def ph_embed():
        st["xp"] = [sb.tile([128, N_I], bf16, tag=f"xp{t}{u}", name=f"xp{t}{u}") for t in range(2)]
        st["xpf"] = [sb.tile([128, N_I], bf16, tag=f"xpf{t}{u}", name=f"xpf{t}{u}") for t in range(2)]
        st["sq"] = [sb.tile([128, N_I], bf16, tag=f"sq{t}{u}", name=f"sq{t}{u}") for t in range(2)]
        cl = slice(u * N_I, (u + 1) * N_I)
        for t in range(2):
            pt = ps.tile([128, N_I], f32, tag="mm", bufs=3)
            nc.tensor.matmul(pt[:, :], W[f"emb_{nm}"][:, 128 * t:128 * (t + 1)],
                             ic[:, cl], start=True, stop=True)
            nc.scalar.activation(st["xpf"][t][:, :], pt[:, :], AF.Copy)
            nc.vector.tensor_copy(st["xp"][t][:, :], pt[:, :])
            nc.scalar.activation(st["sq"][t][:, :], pt[:, :], AF.Square)

    def ph_stats():
        xp, sq = st["xp"], st["sq"]
        rs = sb.tile([128, N_I], f32, tag=f"rs{u}", name=f"rs{u}")
        pm = ps.tile([128, N_I], f32, tag="mm", bufs=3)
        nc.tensor.matmul(pm[:, :], W["ones_ln"][:, :], xp[0][:, :], start=True, stop=False)
        nc.tensor.matmul(pm[:, :], W["ones_ln"][:, :], xp[1][:, :], start=False, stop=True)
        pv = ps.tile([128, N_I], f32, tag="mm", bufs=3)
        nc.tensor.matmul(pv[:, :], W["ones_ln"][:, :], sq[0][:, :], start=True, stop=False)
        nc.tensor.matmul(pv[:, :], W["ones_ln"][:, :], sq[1][:, :], start=False, stop=True)
        t1 = sb.tile([128, N_I], f32, tag=f"t1{u}", name=f"t1{u}")
        nc.scalar.activation(t1[:, :], pm[:, :], AF.Square)
        nc.vector.tensor_sub(t1[:, :], pv[:, :], t1[:, :])
        # rs = rsqrt(var + eps)/16 via fast-inverse-sqrt + 1 Newton step
        t2 = sb.tile([128, N_I], f32, tag=f"t2{u}", name=f"t2{u}")
        y0 = sb.tile([128, N_I], f32, tag=f"y0{u}", name=f"y0{u}")
        LSR = ALU.logical_shift_right
        nc.vector.tensor_scalar(t1[:, :], t1[:, :], LN_EPS, None, ALU.add)
        nc.vector.tensor_scalar(y0[:, :].bitcast(mybir.dt.uint32),
                                t1[:, :].bitcast(mybir.dt.uint32), 1, None, LSR)
        nc.gpsimd.tensor_sub(y0[:, :].bitcast(mybir.dt.uint32),
                             W["magic"][:, :],
                             y0[:, :].bitcast(mybir.dt.uint32))
        nc.gpsimd.tensor_mul(t2[:, :], t1[:, :], y0[:, :])
        nc.gpsimd.tensor_mul(t2[:, :], t2[:, :], y0[:, :])
        nc.vector.tensor_scalar(t2[:, :], t2[:, :], -0.03125, 0.09375, ALU.mult, ALU.add)
        nc.gpsimd.tensor_mul(rs[:, :], y0[:, :], t2[:, :])
        st["rs"] = rs

    def ph_qkv():
        rs = st["rs"]
        cl = slice(u * N_I, (u + 1) * N_I)
        qkv = [sb.tile([128, N_I], bf16, tag=f"qkv{m}{u}", name=f"qkv{m}{u}") for m in range(4)]
        qv = sb.tile([128, 2 * 32 * S_I], bf16, tag=f"qv{u}", name=f"qv{u}")
        for g in range(2):
            nc.gpsimd.memset(
                qv[:, 512 * g:512 * (g + 1)].rearrange("p (s l) -> p s l", l=32)[:, :, L:32], 0.0)
        for m in range(6):
            pq = ps.tile([128, N_I], f32, tag="mm", bufs=3)
            nc.tensor.matmul(pq[:, :], W[f"qkv_{nm}"][:, 128 * m:128 * (m + 1)],
                             ic[:, cl], start=True, stop=True)
            if m in (0, 1):
                nc.vector.tensor_mul(qkv[m][:, :], pq[:, :], rs[:, :])
            elif m in (2, 3):
                nc.vector.tensor_mul(qkv[m][:, :], pq[:, :], rs[:, :])
            else:
                g = m - 4
                nc.vector.tensor_mul(
                    qv[:, 512 * g:512 * (g + 1)].rearrange("p (s l) -> p s l", l=32)[:, :, 0:L],
                    pq[:, :].rearrange("p (s l) -> p s l", l=L),
                    rs[:, :].rearrange("p (s l) -> p s l", l=L))
        st["qkv"] = qkv
        st["qv"] = qv

    def ph_trans():
        vt = sb.tile([128, 2 * 32 * S_I], bf16, tag=f"vt{u}", name=f"vt{u}")
        nc.vector.transpose(vt[:, :], st["qv"][:, :])
        st["vt"] = vt

    def ph_bd():
        vt, qkv = st["vt"], st["qkv"]
        for g in range(2):
            for h in range(4):
                kdst = (kbd[g][32 * h:32 * h + 32, :]
                        .rearrange("p (s m) -> p s m", m=128)[:, :, 32 * h:32 * h + L])
                ksrc = (qkv[2 + g][32 * h:32 * h + 32, :]
                        .rearrange("p (s m) -> p s m", m=L))
                if h != 3:
                    nc.gpsimd.tensor_copy(kdst, ksrc)
                else:
                    nc.scalar.activation(kdst, ksrc, AF.Copy)
        for h in range(4):
            vdst = (vbd[32 * h:32 * h + L, :]
                    .rearrange("p (g s d) -> p g s d", g=2, d=128)[:, :, :, 32 * h:32 * h + 32])
            vsrc = (vt[32 * h:32 * h + L, :]
                    .rearrange("p (g s d) -> p g s d", g=2, d=32))
            if h % 2 == 0:
                nc.scalar.activation(vdst, vsrc, AF.Copy)
            else:
                nc.gpsimd.tensor_copy(vdst, vsrc)

    def ph_attn():
        qkv = st["qkv"]
        avn = [sb.tile([128, N_I], bf16, tag=f"avn{g}{u}", name=f"avn{g}{u}") for g in range(2)]
        for g in range(2):
            sc = ps.tile([128, N_I], f32, tag="sc", bufs=2)
            for j in range(S_I):
                nc.tensor.matmul(
                    sc[0:128, j * L:(j + 1) * L],
                    kbd[g][:, 128 * j:128 * (j + 1)],
                    qkv[g][:, L * j:L * (j + 1)],
                    start=True, stop=True)
            ebuf = sb.tile([128, N_I], bf16, tag=f"e{g}{u}", name=f"e{g}{u}")
            nc.scalar.activation(ebuf[:, :], sc[:, :], AF.Exp, scale=SCALE)
            dn = ps.tile([128, N_I], f32, tag="mm", bufs=3)
            nc.tensor.matmul(dn[:, :], W["ones_den"][:, :], ebuf[:, :],
                             start=True, stop=True)
            rden = sb.tile([128, N_I], f32, tag=f"rden{u}", bufs=1, name=f"rden{u}")
            nc.vector.reciprocal_approx_fast(rden[:, :], dn[:, :])
            av = ps.tile([128, N_I], f32, tag="sc", bufs=2)
            for j in range(S_I):
                nc.tensor.matmul(
                    av[:, j * L:(j + 1) * L],
                    vbd[:, 2048 * g + 128 * j:2048 * g + 128 * (j + 1)],
                    ebuf[:, L * j:L * (j + 1)],
                    start=True, stop=True)
            nc.vector.tensor_mul(avn[g][:, :], av[:, :], rden[:, :])
        st["avn"] = avn

    def ph_proj():
        avn, xpf = st["avn"], st["xpf"]
        cl = slice(u * N_I, (u + 1) * N_I)
        for t in range(2):
            pp = ps.tile([128, N_I], f32, tag="mm", bufs=3)
            nc.tensor.matmul(pp[:, :], W[f"proj_{nm}"][:, 128 * t:128 * (t + 1)],
                             avn[0][:, :], start=True, stop=False)
            nc.tensor.matmul(pp[:, :], W[f"proj_{nm}"][:, 256 + 128 * t:256 + 128 * (t + 1)],
                             avn[1][:, :], start=False, stop=True)
            o2a = sb.tile([128, N_I], f32, tag=f"o2a{t}{u}", name=f"o2a{t}{u}")
            nc.vector.tensor_add(o2a[:, :], pp[:, :], xpf[t][:, :])
            nc.gpsimd.tensor_add(o2[t][:, cl], o2a[:, :],
                                 W[f"fb_{nm}"][:, N_I * t:N_I * (t + 1)])

    return [ph_embed, ph_stats, ph_qkv, ph_trans, ph_bd, ph_attn, ph_proj]


def _build_kernel(nc, tc, b_loc, loop_tok=None, static_dma=False):
    import contextlib
    ctx = contextlib.ExitStack()
    n_tok = b_loc * L
    if loop_tok is None:
        loop_tok = n_tok

    dram = {}
    for nm in ("x", "y"):
        dram[f"ic_{nm}"] = nc.dram_tensor(f"ic_{nm}", [52, n_tok], bf16, kind="ExternalInput").ap()
        dram[f"wemb_{nm}"] = nc.dram_tensor(f"wemb_{nm}", [52, DIM], bf16, kind="ExternalInput").ap()
        dram[f"wqkv_{nm}"] = nc.dram_tensor(f"wqkv_{nm}", [52, 768], bf16, kind="ExternalInput").ap()
        dram[f"wproj_{nm}"] = nc.dram_tensor(f"wproj_{nm}", [128, 512], bf16, kind="ExternalInput").ap()
        dram[f"fb_{nm}"] = nc.dram_tensor(f"fb_{nm}", [128, 2 * N_I], f32, kind="ExternalInput").ap()
    dram["ones_ln"] = nc.dram_tensor("ones_ln", [128, 128], bf16, kind="ExternalInput").ap()
    dram["ones_den"] = nc.dram_tensor("ones_den", [128, 128], bf16, kind="ExternalInput").ap()
    out_d = nc.dram_tensor("out", [2 * DIM, n_tok], bf16, kind="ExternalOutput").ap()

    const = ctx.enter_context(tc.tile_pool(name="const", bufs=1))
    sb = ctx.enter_context(tc.tile_pool(name="sb", bufs=1))
    ps = ctx.enter_context(tc.tile_pool(name="ps", bufs=2, space="PSUM"))

    W = {}
    for nm in ("x", "y"):
        for key, shp, dt in (("emb", [52, DIM], bf16), ("qkv", [52, 768], bf16),
                             ("proj", [128, 512], bf16), ("fb", [128, 2 * N_I], f32)):
            W[f"{key}_{nm}"] = const.tile(shp, dt, tag=f"{key}{nm}", name=f"{key}{nm}")
            nc.sync.dma_start(W[f"{key}_{nm}"][:, :], dram[f"w{key}_{nm}" if key != "fb" else f"fb_{nm}"])
    W["ones_ln"] = const.tile([128, 128], bf16, tag="ones_ln", name="ones_ln")
    nc.sync.dma_start(W["ones_ln"][:, :], dram["ones_ln"])
    W["ones_den"] = const.tile([128, 128], bf16, tag="ones_den", name="ones_den")
    nc.sync.dma_start(W["ones_den"][:, :], dram["ones_den"])
    W["eps256"] = const.tile([128, 1], f32, tag="eps256", name="eps256")
    nc.vector.memset(W["eps256"][:, :], 256.0 * LN_EPS)
    W["magic"] = const.tile([128, N_I], mybir.dt.uint32, tag="magic", name="magic")
    nc.vector.memset(W["magic"][:, :], 0x5f3759df)

    # block-diag staging tiles: preamble-zeroed once; loop bodies overwrite
    # only the in-block 25/32-col regions, padding stays zero.
    kbd, vbd = {}, {}
    for u in range(U):
        for g in range(2):
            kbd[(g, u)] = const.tile([128, 128 * S_I], bf16, tag=f"kbd{g}{u}", name=f"kbd{g}{u}")
            nc.vector.memset(kbd[(g, u)][:, :], 0.0)
        vbd[u] = const.tile([128, 2 * 128 * S_I], bf16, tag=f"vbd{u}", name=f"vbd{u}")
        nc.vector.memset(vbd[u][:, :], 0.0)

    for nm in ("x", "y"):
        ob = 0 if nm == "x" else DIM
        with tc.For_i(0, loop_tok, TOK_B, name=f"chunks_{nm}", staggered_reset=True,
                      hint_engines=(mybir.EngineType.PE,)) as tok0:
            ic = sb.tile([52, TOK_B], bf16, tag="ic", bufs=2)
            if static_dma:
                nc.sync.dma_start(ic[:, :], dram[f"ic_{nm}"][:, 0:TOK_B])
            else:
                nc.sync.dma_start(ic[:, :], dram[f"ic_{nm}"][:, ds(tok0, TOK_B)])
            o2 = [sb.tile([128, TOK_B], bf16, tag=f"o2{t}", bufs=2, name=f"o2{t}") for t in range(2)]
            phases = [_slice_phases(nc, sb, ps, W, nm, ic, u,
                                    [kbd[(0, u)], kbd[(1, u)]], vbd[u], o2)
                      for u in range(U)]
            n_ph = len(phases[0])
            for slot in range(n_ph + STAGGER * (U - 1)):
                for u in range(U):
                    p = slot - STAGGER * u
                    if 0 <= p < n_ph:
                        phases[u][p]()
            for t in range(2):
                if static_dma:
                    nc.sync.dma_start(out_d[ob + 128 * t: ob + 128 * (t + 1), 0:TOK_B],
                                      o2[t][:, :])
                else:
                    nc.sync.dma_start(out_d[ob + 128 * t: ob + 128 * (t + 1), ds(tok0, TOK_B)],
                                      o2[t][:, :])
    ctx.close()


def _get_nc(b_loc, loop_tok=None, static_dma=False):
    key = (b_loc, loop_tok, static_dma)
    if key in _CACHE:
        return _CACHE[key]
    nc = bacc.Bacc("TRN2", target_bir_lowering=False, debug=False,
                   enable_asserts=False, num_devices=NCORES)
    with tile.TileContext(nc, trace_sim=False) as tc:
        _build_kernel(nc, tc, b_loc, loop_tok, static_dma)
    nc.compile()
    bass.Bass.finalize(nc)
    _CACHE[key] = nc
    return nc


def _in_maps(prep, b_loc, ncores):
    maps = []
    for c in range(ncores):
        s0 = c * b_loc
        m = {}
        for nm in ("x", "y"):
            ic = prep[f"ic_{nm}"][s0:s0 + b_loc].reshape(b_loc * L, 52).T
            m[f"ic_{nm}"] = _to_bf16(np.ascontiguousarray(ic))
            m[f"wemb_{nm}"] = _to_bf16(prep[f"wemb_{nm}"])
            m[f"wqkv_{nm}"] = _to_bf16(prep[f"wqkv_{nm}"])
            m[f"wproj_{nm}"] = _to_bf16(prep[f"wproj_{nm}"])
            m[f"fb_{nm}"] = prep[f"fb_{nm}"].astype(np.float32)
        m["ones_ln"] = _to_bf16(prep["ones_ln"])
        m["ones_den"] = _to_bf16(prep["ones_den"])
        maps.append(m)
    return maps


def kernel(**inputs):
    prep = _host_prep(inputs)
    nc = _get_nc(B_LOC)
    res = bass_utils.run_bass_kernel_spmd(nc, _in_maps(prep, B_LOC, NCORES),
                                          core_ids=list(range(NCORES)))
    outs = [res.results[c]["out"] for c in range(NCORES)]
    full = np.concatenate(
        [np.asarray(o, np.float32).reshape(2 * DIM, B_LOC, L).transpose(1, 0, 2)
         for o in outs], axis=0)
    return np.ascontiguousarray(full.reshape(B, 2 * DIM, 5, 5))


# revision 27
# speedup vs baseline: 1743.1385x; 1.0032x over previous
"""Trainium2 Bass kernel for nn_Block_CD (dual-stream patch-embed + attention).

Math per stream (x / y), tokens = (sample, l), l = 25 positions:
  xp  = conv3x3(img) + pos + conv_b           (im2col-ext matmul, K=52)
  xln = (xp - mu) * rstd                      (LN; g/b folded into qkv weights)
  qkv = (16 * qkv_w * g).T @ xln              (feature-major [768, tok])
  scores^T[(h,m),l] = Kbd_s^T @ q_s           (block-diag 4-head groups)
  E = exp(SCALE * scores); den = O^T @ E      (replicated over (h,d))
  av = Vbd_s^T @ E; avn = av / den
  out = proj^T @ avn + xp + (bp - pos - conv_b)
Output device layout: [512, B_loc*25] bf16, rearranged on host.
Sharding: pure data parallel, B=8192 over 8 cores.

Final structure (v8):
- hardware For_i loop (staggered_reset) over 1600-token chunks, 4 slices
  of 16 samples software-pipelined inside each body (phase stagger=1);
  33k-instruction unrolled baseline -> ~2.7k BIR instructions, which
  removes the dominant per-call host lowering/dispatch cost.
- LN mean fold: qkv = A @ im2col with A = (Wemb - mean) @ Wqkv^T, so the
  qkv matmuls contract 52 instead of 256 and need no centered activations.
- rstd via fast-inverse-sqrt bit trick + 1 Newton step (DVE/Pool), so the
  Activation engine runs a single function table ({Copy,Square,Exp}) and
  never pays the ~1.3us table-reload inside the loop.
- engine placement respects HW limits (GPSIMD cannot read PSUM; Pool has
  no TensorScalarPtr): PSUM consumers on ACT/DVE, SBUF-only block-diag
  builds and the residual+bias add on Pool.
"""
import sys
sys.path.insert(0, "/opt/trn_rl_repo")
import numpy as np
import ml_dtypes

import concourse.bass as bass
import concourse.mybir as mybir
import concourse.tile as tile
from concourse import bacc, bass_utils
from concourse.bass import ds

bf16 = mybir.dt.bfloat16
f32 = mybir.dt.float32
AF = mybir.ActivationFunctionType
ALU = mybir.AluOpType

DIM = 256
HEADS = 8
HD = 32
L = 25
SCALE = HD ** -0.5
LN_EPS = 1e-5
NCORES = 8
B = 8192
B_LOC = B // NCORES

S_I = 16          # samples per inner psum slice
N_I = S_I * L     # 400
U = 4             # slices unrolled per hw-loop iteration
STAGGER = 1       # phase offset between consecutive slices (sw pipeline)
TOK_B = U * N_I   # 800 tokens per iteration

_CACHE = {}


def _to_bf16(a):
    return np.asarray(a, np.float32).astype(ml_dtypes.bfloat16)


def _host_prep(inputs):
    pos = np.asarray(inputs["pos_embed"], np.float32).reshape(L, DIM)
    ln_g = np.asarray(inputs["ln_g"], np.float32)
    ln_b = np.asarray(inputs["ln_b"], np.float32)

    def im2col_ext(img):
        p = np.pad(np.asarray(img, np.float32), ((0, 0), (0, 0), (1, 1), (1, 1)))
        Bn = img.shape[0]
        cols = np.empty((Bn, L, 52), np.float32)
        idx = 0
        for c in range(3):
            for di in range(3):
                for dj in range(3):
                    cols[:, :, idx] = p[:, c, di:di + 5, dj:dj + 5].reshape(Bn, L)
                    idx += 1
        cols[:, :, 27:] = np.eye(L, dtype=np.float32)[None]
        return cols  # [B, 25, 52]

    prep = {}
    for nm, ik, cw, cb, qw, pw, pb in (
        ("x", "x", "conv1_w", "conv1_b", "qkv_x_w", "proj_x_w", "proj_x_b"),
        ("y", "y", "conv2_w", "conv2_b", "qkv_y_w", "proj_y_w", "proj_y_b"),
    ):
        conv_w = np.asarray(inputs[cw], np.float32)
        conv_b = np.asarray(inputs[cb], np.float32)
        qkv_w = np.asarray(inputs[qw], np.float32)
        proj_w = np.asarray(inputs[pw], np.float32)
        proj_b = np.asarray(inputs[pb], np.float32)

        w_emb = np.empty((52, DIM), np.float32)
        w_emb[:27] = conv_w.reshape(DIM, 27).T
        w_emb[27:] = pos + conv_b[None, :]
        prep[f"wemb_{nm}"] = w_emb

        wq = (16.0 * qkv_w * ln_g[None, :]).T          # [256, 768]
        w_emb_c = w_emb - w_emb.mean(axis=1, keepdims=True)
        prep[f"wqkv_{nm}"] = w_emb_c @ wq              # [52, 768] (LN mean folded)
        c = qkv_w @ ln_b
        assert np.abs(c).max() < 1e-6, "nonzero ln_b fold not supported"

        wp = proj_w.T                                   # [256, 256] lhsT
        prep[f"wproj_{nm}"] = np.concatenate([wp[0:128], wp[128:256]], axis=1)  # [128,512]

        fb = proj_b[:, None] - pos.T - conv_b[:, None]  # [256, 25]
        fbt = np.tile(fb, (1, S_I))                     # [256, 400]
        prep[f"fb_{nm}"] = np.concatenate([fbt[0:128], fbt[128:256]], axis=1)  # [128,800]

        prep[f"ic_{nm}"] = im2col_ext(inputs[ik])

    prep["ones_ln"] = np.full((128, 128), 1.0 / DIM, np.float32)
    O = np.zeros((128, 128), np.float32)
    for h in range(4):
        O[h * HD:h * HD + L, h * HD:(h + 1) * HD] = 1.0
    prep["ones_den"] = O
    return prep


def _slice_phases(nc, sb, ps, W, nm, ic, u, kbd, vbd, o2):
    """Return a list of phase-emitter closures for one 16-sample slice."""
    st = {}

    def ph_embed():
        st["xp"] = [sb.tile([128, N_I], bf16, tag=f"xp{t}{u}", name=f"xp{t}{u}") for t in range(2)]
        st["xpf"] = [sb.tile([128, N_I], bf16, tag=f"xpf{t}{u}", name=f"xpf{t}{u}") for t in range(2)]
        st["sq"] = [sb.tile([128, N_I], bf16, tag=f"sq{t}{u}", name=f"sq{t}{u}") for t in range(2)]
        cl = slice(u * N_I, (u + 1) * N_I)
        for t in range(2):
            pt = ps.tile([128, N_I], f32, tag="mm", bufs=4)
            nc.tensor.matmul(pt[:, :], W[f"emb_{nm}"][:, 128 * t:128 * (t + 1)],
                             ic[:, cl], start=True, stop=True)
            nc.scalar.activation(st["xpf"][t][:, :], pt[:, :], AF.Copy)
            nc.vector.tensor_copy(st["xp"][t][:, :], pt[:, :])
            nc.scalar.activation(st["sq"][t][:, :], pt[:, :], AF.Square)

    def ph_stats():
        xp, sq = st["xp"], st["sq"]
        rs = sb.tile([128, N_I], f32, tag=f"rs{u}", name=f"rs{u}")
        pm = ps.tile([128, N_I], f32, tag="mm", bufs=4)
        nc.tensor.matmul(pm[:, :], W["ones_ln"][:, :], xp[0][:, :], start=True, stop=False)
        nc.tensor.matmul(pm[:, :], W["ones_ln"][:, :], xp[1][:, :], start=False, stop=True)
        pv = ps.tile([128, N_I], f32, tag="mm", bufs=4)
        nc.tensor.matmul(pv[:, :], W["ones_ln"][:, :], sq[0][:, :], start=True, stop=False)
        nc.tensor.matmul(pv[:, :], W["ones_ln"][:, :], sq[1][:, :], start=False, stop=True)
        t1 = sb.tile([128, N_I], f32, tag=f"t1{u}", name=f"t1{u}")
        nc.scalar.activation(t1[:, :], pm[:, :], AF.Square)
        nc.vector.tensor_sub(t1[:, :], pv[:, :], t1[:, :])
        # rs = rsqrt(var + eps)/16 via fast-inverse-sqrt + 1 Newton step
        t2 = sb.tile([128, N_I], f32, tag=f"t2{u}", name=f"t2{u}")
        y0 = sb.tile([128, N_I], f32, tag=f"y0{u}", name=f"y0{u}")
        LSR = ALU.logical_shift_right
        nc.vector.tensor_scalar(t1[:, :], t1[:, :], LN_EPS, None, ALU.add)
        nc.vector.tensor_scalar(y0[:, :].bitcast(mybir.dt.uint32),
                                t1[:, :].bitcast(mybir.dt.uint32), 1, None, LSR)
        nc.gpsimd.tensor_sub(y0[:, :].bitcast(mybir.dt.uint32),
                             W["magic"][:, :],
                             y0[:, :].bitcast(mybir.dt.uint32))
        nc.gpsimd.tensor_mul(t2[:, :], t1[:, :], y0[:, :])
        nc.gpsimd.tensor_mul(t2[:, :], t2[:, :], y0[:, :])
        nc.vector.tensor_scalar(t2[:, :], t2[:, :], -0.03125, 0.09375, ALU.mult, ALU.add)
        nc.gpsimd.tensor_mul(rs[:, :], y0[:, :], t2[:, :])
        st["rs"] = rs

    def ph_qkv():
        rs = st["rs"]
        cl = slice(u * N_I, (u + 1) * N_I)
        qkv = [sb.tile([128, N_I], bf16, tag=f"qkv{m}{u}", name=f"qkv{m}{u}") for m in range(4)]
        qv = sb.tile([128, 2 * 32 * S_I], bf16, tag=f"qv{u}", name=f"qv{u}")
        for g in range(2):
            nc.gpsimd.memset(
                qv[:, 512 * g:512 * (g + 1)].rearrange("p (s l) -> p s l", l=32)[:, :, L:32], 0.0)
        for m in range(6):
            pq = ps.tile([128, N_I], f32, tag="mm", bufs=4)
            nc.tensor.matmul(pq[:, :], W[f"qkv_{nm}"][:, 128 * m:128 * (m + 1)],
                             ic[:, cl], start=True, stop=True)
            if m in (0, 1):
                nc.vector.tensor_mul(qkv[m][:, :], pq[:, :], rs[:, :])
            elif m in (2, 3):
                nc.vector.tensor_mul(qkv[m][:, :], pq[:, :], rs[:, :])
            else:
                g = m - 4
                nc.vector.tensor_mul(
                    qv[:, 512 * g:512 * (g + 1)].rearrange("p (s l) -> p s l", l=32)[:, :, 0:L],
                    pq[:, :].rearrange("p (s l) -> p s l", l=L),
                    rs[:, :].rearrange("p (s l) -> p s l", l=L))
        st["qkv"] = qkv
        st["qv"] = qv

    def ph_trans():
        vt = sb.tile([128, 2 * 32 * S_I], bf16, tag=f"vt{u}", name=f"vt{u}")
        nc.vector.transpose(vt[:, :], st["qv"][:, :])
        st["vt"] = vt

    def ph_bd():
        vt, qkv = st["vt"], st["qkv"]
        for g in range(2):
            for h in range(4):
                kdst = (kbd[g][32 * h:32 * h + 32, :]
                        .rearrange("p (s m) -> p s m", m=128)[:, :, 32 * h:32 * h + L])
                ksrc = (qkv[2 + g][32 * h:32 * h + 32, :]
                        .rearrange("p (s m) -> p s m", m=L))
                if h != 3:
                    nc.gpsimd.tensor_copy(kdst, ksrc)
                else:
                    nc.scalar.activation(kdst, ksrc, AF.Copy)
        for h in range(4):
            vdst = (vbd[32 * h:32 * h + L, :]
                    .rearrange("p (g s d) -> p g s d", g=2, d=128)[:, :, :, 32 * h:32 * h + 32])
            vsrc = (vt[32 * h:32 * h + L, :]
                    .rearrange("p (g s d) -> p g s d", g=2, d=32))
            if h % 2 == 0:
                nc.scalar.activation(vdst, vsrc, AF.Copy)
            else:
                nc.gpsimd.tensor_copy(vdst, vsrc)

    def ph_attn():
        qkv = st["qkv"]
        avn = [sb.tile([128, N_I], bf16, tag=f"avn{g}{u}", name=f"avn{g}{u}") for g in range(2)]
        for g in range(2):
            sc = ps.tile([128, N_I], f32, tag="sc", bufs=3)
            for j in range(S_I):
                nc.tensor.matmul(
                    sc[0:128, j * L:(j + 1) * L],
                    kbd[g][:, 128 * j:128 * (j + 1)],
                    qkv[g][:, L * j:L * (j + 1)],
                    start=True, stop=True)
            ebuf = sb.tile([128, N_I], bf16, tag=f"e{g}{u}", name=f"e{g}{u}")
            nc.scalar.activation(ebuf[:, :], sc[:, :], AF.Exp, scale=SCALE)
            dn = ps.tile([128, N_I], f32, tag="mm", bufs=4)
            nc.tensor.matmul(dn[:, :], W["ones_den"][:, :], ebuf[:, :],
                             start=True, stop=True)
            rden = sb.tile([128, N_I], f32, tag=f"rden{u}", bufs=1, name=f"rden{u}")
            nc.vector.reciprocal_approx_fast(rden[:, :], dn[:, :])
            av = ps.tile([128, N_I], f32, tag="sc", bufs=3)
            for j in range(S_I):
                nc.tensor.matmul(
                    av[:, j * L:(j + 1) * L],
                    vbd[:, 2048 * g + 128 * j:2048 * g + 128 * (j + 1)],
                    ebuf[:, L * j:L * (j + 1)],
                    start=True, stop=True)
            nc.vector.tensor_mul(avn[g][:, :], av[:, :], rden[:, :])
        st["avn"] = avn

    def ph_proj():
        avn, xpf = st["avn"], st["xpf"]
        cl = slice(u * N_I, (u + 1) * N_I)
        for t in range(2):
            pp = ps.tile([128, N_I], f32, tag="mm", bufs=4)
            nc.tensor.matmul(pp[:, :], W[f"proj_{nm}"][:, 128 * t:128 * (t + 1)],
                             avn[0][:, :], start=True, stop=False)
            nc.tensor.matmul(pp[:, :], W[f"proj_{nm}"][:, 256 + 128 * t:256 + 128 * (t + 1)],
                             avn[1][:, :], start=False, stop=True)
            o2a = sb.tile([128, N_I], f32, tag=f"o2a{t}{u}", name=f"o2a{t}{u}")
            nc.vector.tensor_add(o2a[:, :], pp[:, :], xpf[t][:, :])
            nc.gpsimd.tensor_add(o2[t][:, cl], o2a[:, :],
                                 W[f"fb_{nm}"][:, N_I * t:N_I * (t + 1)])

    return [ph_embed, ph_stats, ph_qkv, ph_trans, ph_bd, ph_attn, ph_proj]


def _build_kernel(nc, tc, b_loc, loop_tok=None, static_dma=False):
    import contextlib
    ctx = contextlib.ExitStack()
    n_tok = b_loc * L
    if loop_tok is None:
        loop_tok = n_tok

    dram = {}
    for nm in ("x", "y"):
        dram[f"ic_{nm}"] = nc.dram_tensor(f"ic_{nm}", [52, n_tok], bf16, kind="ExternalInput").ap()
        dram[f"wemb_{nm}"] = nc.dram_tensor(f"wemb_{nm}", [52, DIM], bf16, kind="ExternalInput").ap()
        dram[f"wqkv_{nm}"] = nc.dram_tensor(f"wqkv_{nm}", [52, 768], bf16, kind="ExternalInput").ap()
        dram[f"wproj_{nm}"] = nc.dram_tensor(f"wproj_{nm}", [128, 512], bf16, kind="ExternalInput").ap()
        dram[f"fb_{nm}"] = nc.dram_tensor(f"fb_{nm}", [128, 2 * N_I], f32, kind="ExternalInput").ap()
    dram["ones_ln"] = nc.dram_tensor("ones_ln", [128, 128], bf16, kind="ExternalInput").ap()
    dram["ones_den"] = nc.dram_tensor("ones_den", [128, 128], bf16, kind="ExternalInput").ap()
    out_d = nc.dram_tensor("out", [2 * DIM, n_tok], bf16, kind="ExternalOutput").ap()

    const = ctx.enter_context(tc.tile_pool(name="const", bufs=1))
    sb = ctx.enter_context(tc.tile_pool(name="sb", bufs=1))
    ps = ctx.enter_context(tc.tile_pool(name="ps", bufs=2, space="PSUM"))

    W = {}
    for nm in ("x", "y"):
        for key, shp, dt in (("emb", [52, DIM], bf16), ("qkv", [52, 768], bf16),
                             ("proj", [128, 512], bf16), ("fb", [128, 2 * N_I], f32)):
            W[f"{key}_{nm}"] = const.tile(shp, dt, tag=f"{key}{nm}", name=f"{key}{nm}")
            nc.sync.dma_start(W[f"{key}_{nm}"][:, :], dram[f"w{key}_{nm}" if key != "fb" else f"fb_{nm}"])
    W["ones_ln"] = const.tile([128, 128], bf16, tag="ones_ln", name="ones_ln")
    nc.sync.dma_start(W["ones_ln"][:, :], dram["ones_ln"])
    W["ones_den"] = const.tile([128, 128], bf16, tag="ones_den", name="ones_den")
    nc.sync.dma_start(W["ones_den"][:, :], dram["ones_den"])
    W["eps256"] = const.tile([128, 1], f32, tag="eps256", name="eps256")
    nc.vector.memset(W["eps256"][:, :], 256.0 * LN_EPS)
    W["magic"] = const.tile([128, N_I], mybir.dt.uint32, tag="magic", name="magic")
    nc.vector.memset(W["magic"][:, :], 0x5f3759df)

    # block-diag staging tiles: preamble-zeroed once; loop bodies overwrite
    # only the in-block 25/32-col regions, padding stays zero.
    kbd, vbd = {}, {}
    for u in range(U):
        for g in range(2):
            kbd[(g, u)] = const.tile([128, 128 * S_I], bf16, tag=f"kbd{g}{u}", name=f"kbd{g}{u}")
            nc.vector.memset(kbd[(g, u)][:, :], 0.0)
        vbd[u] = const.tile([128, 2 * 128 * S_I], bf16, tag=f"vbd{u}", name=f"vbd{u}")
        nc.vector.memset(vbd[u][:, :], 0.0)

    for nm in ("x", "y"):
        ob = 0 if nm == "x" else DIM
        with tc.For_i(0, loop_tok, TOK_B, name=f"chunks_{nm}", staggered_reset=True,
                      hint_engines=(mybir.EngineType.PE,)) as tok0:
            ic = sb.tile([52, TOK_B], bf16, tag="ic", bufs=2)
            if static_dma:
                nc.sync.dma_start(ic[:, :], dram[f"ic_{nm}"][:, 0:TOK_B])
            else:
                nc.sync.dma_start(ic[:, :], dram[f"ic_{nm}"][:, ds(tok0, TOK_B)])
            o2 = [sb.tile([128, TOK_B], bf16, tag=f"o2{t}", bufs=2, name=f"o2{t}") for t in range(2)]
            phases = [_slice_phases(nc, sb, ps, W, nm, ic, u,
                                    [kbd[(0, u)], kbd[(1, u)]], vbd[u], o2)
                      for u in range(U)]
            n_ph = len(phases[0])
            for slot in range(n_ph + STAGGER * (U - 1)):
                for u in range(U):
                    p = slot - STAGGER * u
                    if 0 <= p < n_ph:
                        phases[u][p]()
            for t in range(2):
                if static_dma:
                    nc.sync.dma_start(out_d[ob + 128 * t: ob + 128 * (t + 1), 0:TOK_B],
                                      o2[t][:, :])
                else:
                    nc.sync.dma_start(out_d[ob + 128 * t: ob + 128 * (t + 1), ds(tok0, TOK_B)],
                                      o2[t][:, :])
    ctx.close()


def _get_nc(b_loc, loop_tok=None, static_dma=False):
    key = (b_loc, loop_tok, static_dma)
    if key in _CACHE:
        return _CACHE[key]
    nc = bacc.Bacc("TRN2", target_bir_lowering=False, debug=False,
                   enable_asserts=False, num_devices=NCORES)
    with tile.TileContext(nc, trace_sim=False) as tc:
        _build_kernel(nc, tc, b_loc, loop_tok, static_dma)
    nc.compile()
    bass.Bass.finalize(nc)
    _CACHE[key] = nc
    return nc


def _in_maps(prep, b_loc, ncores):
    maps = []
    for c in range(ncores):
        s0 = c * b_loc
        m = {}
        for nm in ("x", "y"):
            ic = prep[f"ic_{nm}"][s0:s0 + b_loc].reshape(b_loc * L, 52).T
            m[f"ic_{nm}"] = _to_bf16(np.ascontiguousarray(ic))
            m[f"wemb_{nm}"] = _to_bf16(prep[f"wemb_{nm}"])
            m[f"wqkv_{nm}"] = _to_bf16(prep[f"wqkv_{nm}"])
            m[f"wproj_{nm}"] = _to_bf16(prep[f"wproj_{nm}"])
            m[f"fb_{nm}"] = prep[f"fb_{nm}"].astype(np.float32)
        m["ones_ln"] = _to_bf16(prep["ones_ln"])
        m["ones_den"] = _to_bf16(prep["ones_den"])
        maps.append(m)
    return maps


def kernel(**inputs):
    prep = _host_prep(inputs)
    nc = _get_nc(B_LOC)
    res = bass_utils.run_bass_kernel_spmd(nc, _in_maps(prep, B_LOC, NCORES),
                                          core_ids=list(range(NCORES)))
    outs = [res.results[c]["out"] for c in range(NCORES)]
    full = np.concatenate(
        [np.asarray(o, np.float32).reshape(2 * DIM, B_LOC, L).transpose(1, 0, 2)
         for o in outs], axis=0)
    return np.ascontiguousarray(full.reshape(B, 2 * DIM, 5, 5))
